# revision 26
# baseline (speedup 1.0000x reference)
"""Trainium2 Bass kernel for nn_Network_67388036874689.

Data-parallel over batch: B=256 as 32 samples on each of 8 cores; params
replicated (host-precomposed).

Structure exploited (validated against the reference on host):
  - fog_of_war's greedy scan returns arange(B) -> permutation is identity.
  - conv2d(3x3, pad=1) on [C, H, 1] only sees kernel column 1 -> 1D 3-tap
    conv over H.
  - THE BIG FUSION: embedding + pair-maxpool + conv + linear collapse into
    a single gathered table per branch:
        logits[b, j] = sum_h G[pid[b, h], h, j]
    with G[p, h, j] = sum_kh CW_kh[p, :] @ W[:, h+1-kh, j] (host-built,
    boundary-clipped), pid = canonical unordered pair index (105 rows,
    pair-max is symmetric).  Linear bias folds in as bias[j]/128.
  - On device each branch: one-hot(pid) built 512 cols at a time
    (ones-matmul broadcast + is_equal vs iota), then 128 accumulating
    matmuls lpsum[32, J] += OH[:, h]^T @ G[:, h, :], 4-way col-tiled
    (tile_position=(0,32*(h%4))) so 4 streams share the PE array.
  - Both tables fp8 e4m3 (scaled; descale folded into the Exp scales):
    host sim rel err ~1.2e-3 vs 2e-2 tolerance.  The per-core DMA path
    sustains only ~200 GB/s (8 cores share HBM), so bytes are the
    enemy-phase currency: GE is streamed as 4 chunks with 4KB rows
    (per-descriptor packet overhead ~100ns amortizes with row length),
    in consumption order on the sync ring.
  - Manipulator fully bf16 (host sim 1.2e-3): conv input constant over
    h -> collapses to [128,192] wsumT + stacked [128/64, 256] linears.
  - floor via the 2^23 round-down magic (+max clamp for t<0.25); mod 14
    via 2 conditional subtracts; friend pair index = canonical sym index
    via ALU min/max: idx = hi + 13.5*lo - 0.5*lo^2.
"""

import numpy as np
import ml_dtypes
from contextlib import ExitStack

import concourse.bass as bass
import concourse.bacc as bacc
import concourse.mybir as mybir
import concourse.tile as tile
from concourse.masks import make_identity
from concourse.bass_utils import run_bass_kernel_spmd

F32 = mybir.dt.float32
F32R = mybir.dt.float32r
BF16 = mybir.dt.bfloat16
FP8 = mybir.dt.float8e4
I32 = mybir.dt.int32
AF = mybir.ActivationFunctionType
ALU = mybir.AluOpType
AX = mybir.AxisListType

NCORES = 8
B = 256
BC = B // NCORES        # 32 samples per core
L = 256
V = 14
H = L // 2              # 128 pooled positions
NSYM = V * (V + 1) // 2  # 105 canonical pairs
DEBUG_TAPS = False


def _dram_inputs(nc):
    t = {}

    def inp(name, shape, dt):
        t[name] = nc.dram_tensor(name, list(shape), dt, kind="ExternalInput").ap()

    inp("idxrowE", (1, BC * H), BF16)    # h-major: col = h*32 + b
    inp("GE", (4 * NSYM, 4096), FP8)     # chunk-major: chunk c rows hold
                                         # GE blocks 2c | 2c+1 (4KB rows)
    inp("GF", (NSYM, H * 14), FP8)       # col = h*14 + n, bias folded, *sF
    inp("pack", (128, 704), BF16)        # wsumT[0:192] mlwA[192:448]
                                         # mlwB[448:704] (rows 64+ zero)
    inp("mcb2", (128,), F32)
    t["out"] = nc.dram_tensor("out", [BC, 14], F32, kind="ExternalOutput").ap()
    return t


def _tap(nc, io, name, ap):
    if not DEBUG_TAPS:
        return
    t = nc.dram_tensor("tap_" + name, list(ap.shape), ap.dtype,
                       kind="ExternalOutput").ap()
    io["tap_" + name] = t
    nc.gpsimd.dma_start(t, ap)


def build_kernel(nc, tc, ctx, inv_se, inv_sf):
    io = _dram_inputs(nc)
    consts = ctx.enter_context(tc.tile_pool(name="consts", bufs=1))
    work = ctx.enter_context(tc.tile_pool(name="work", bufs=1))
    gepool = ctx.enter_context(tc.tile_pool(name="gepool", bufs=4))
    ohpool = ctx.enter_context(tc.tile_pool(name="ohpool", bufs=8))
    ppp = ctx.enter_context(tc.tile_pool(name="ppp", bufs=3, space="PSUM"))
    pacc = ctx.enter_context(tc.tile_pool(name="pacc", bufs=1, space="PSUM"))
    psm = ctx.enter_context(tc.tile_pool(name="psm", bufs=1, space="PSUM"))
    cxp = ctx.enter_context(tc.tile_pool(name="cxp", bufs=2, space="PSUM"))

    def ctile(shape, dt, tag):
        return consts.tile(shape, dt, tag=tag, name=tag)

    def wtile(shape, dt, tag):
        return work.tile(shape, dt, tag=tag, name=tag)

    # ---------------- constants ----------------
    identF = ctile([128, 128], F32, "identF")
    make_identity(nc, identF)
    iota_i = ctile([128, 1], I32, "iota_i")
    nc.gpsimd.iota(iota_i[:, :], pattern=[[0, 1]], base=0, channel_multiplier=1)
    iota_col = ctile([128, 1], F32, "iota_col")
    nc.vector.tensor_copy(iota_col[:, :], iota_i[:, :])
    ones_row = ctile([1, NSYM], BF16, "ones_row")
    nc.vector.memset(ones_row[:, :], 1.0)
    cxshL = ctile([65, BC], BF16, "cxshL")
    nc.vector.memset(cxshL[64:65, :], 1.0)   # mlb row of the augmented GEMM

    # sync ring, consumption order: idx row, GE chunks, GF
    idxrowE = wtile([1, BC * H], BF16, "idxrowE")
    nc.sync.dma_start(idxrowE[:, :], io["idxrowE"])
    GEc = []
    for c in range(4):
        t = gepool.tile([NSYM, 4096], FP8, tag="gec", name=f"gec{c}")
        nc.sync.dma_start(t[:, :], io["GE"][c * NSYM:(c + 1) * NSYM, :])
        GEc.append(t)
    GF = ctile([NSYM, H * 14], FP8, "GF")
    nc.sync.dma_start(GF[:, :], io["GF"])

    # small weights on the gpsimd ring
    mcb_col = ctile([128, 1], F32, "mcb2")
    nc.gpsimd.dma_start(mcb_col[:, :], io["mcb2"])
    pack = ctile([128, 704], BF16, "pack")
    nc.gpsimd.dma_start(pack[:, :], io["pack"])

    # ---------------- shared branch machinery ----------------
    def branch(idxrow, Gslice, J, lptile, tag, ways, frontload=False):
        """For each 16-h block: broadcast idx (PE), one-hot (DVE is_eq,
        fp8), then 16 gather matmuls col-tiled `ways`-wide into lptile.
        frontload issues all 8 pp+is_eq pairs before any gather so the
        one-hot build overlaps the table DMA window instead of trailing
        the gathers in PE program order."""
        def onehot(k):
            pp = ppp.tile([NSYM, 512], F32, tag="pp", name="pp")
            nc.tensor.matmul(pp[:, :], ones_row[:, :],
                             idxrow[:, k * 512:(k + 1) * 512],
                             start=True, stop=True)
            oh = ohpool.tile([NSYM, 512], FP8, tag="oh", name=f"oh{tag}{k}")
            nc.vector.tensor_scalar(oh[:, :], pp[:, :], iota_col[0:NSYM, :],
                                    None, ALU.is_equal)
            return oh

        def gathers(k, oh):
            for hh in range(16):
                h = k * 16 + hh
                j = h % ways
                nc.tensor.matmul(
                    lptile[32 * j:32 * (j + 1), 0:J],
                    oh[:, hh * 32:(hh + 1) * 32],
                    Gslice(k, hh),
                    start=(h < ways), stop=(h >= 128 - ways),
                    tile_position=(0, 32 * j), skip_group_check=True)

        if frontload:
            ohs = [onehot(k) for k in range(8)]
            for k in range(8):
                gathers(k, ohs[k])
        else:
            for k in range(8):
                gathers(k, onehot(k))

    def combine4(lptile, J, tag):
        """Sum the 4 col-tile partials (one PSUM read per DVE op)."""
        c1 = wtile([BC, J], F32, f"c1{tag}")
        nc.vector.tensor_copy(c1[:, :], lptile[32:64, 0:J])
        c2 = wtile([BC, J], F32, f"c2{tag}")
        nc.vector.tensor_copy(c2[:, :], lptile[64:96, 0:J])
        c3 = wtile([BC, J], F32, f"c3{tag}")
        nc.vector.tensor_copy(c3[:, :], lptile[96:128, 0:J])
        a01 = wtile([BC, J], F32, f"a01{tag}")
        nc.vector.tensor_tensor(a01[:, :], lptile[0:32, 0:J], c1[:, :], ALU.add)
        a23 = wtile([BC, J], F32, f"a23{tag}")
        nc.vector.tensor_tensor(a23[:, :], c2[:, :], c3[:, :], ALU.add)
        lg = wtile([BC, J], F32, f"lg{tag}")
        nc.vector.tensor_tensor(lg[:, :], a01[:, :], a23[:, :], ALU.add)
        return lg

    # ---------------- enemy branch ----------------
    lpE = pacc.tile([64, 128], F32, tag="lp", name="lpE")
    branch(idxrowE,
           lambda k, hh: GEc[k // 2][:, (k % 2) * 2048 + hh * 128:
                                     (k % 2) * 2048 + (hh + 1) * 128],
           128, lpE, "E", 2, frontload=True)
    cE1 = wtile([BC, 128], F32, "cE1")
    nc.vector.tensor_copy(cE1[:, :], lpE[32:64, :])
    logitsE = wtile([BC, 128], F32, "logitsE")
    nc.vector.tensor_tensor(logitsE[:, :], lpE[0:32, :], cE1[:, :], ALU.add)
    _tap(nc, io, "logitsE", logitsE[:, :])
    ExE = wtile([BC, 128], F32, "ExE")
    smE = wtile([BC, 1], F32, "smE")
    nc.scalar.activation(ExE[:, :], logitsE[:, :], AF.Exp, scale=float(inv_se),
                         accum_out=smE[:, :])
    rsE = wtile([BC, 1], F32, "rsE")
    nc.vector.reciprocal(rsE[:, :], smE[:, :])
    eout = wtile([BC, 128], F32, "eout")
    nc.vector.tensor_scalar(eout[:, :], ExE[:, :], rsE[:, :], None, ALU.mult)
    _tap(nc, io, "eout", eout[:, :])

    # ---------------- manipulator (bf16) ----------------
    tpv = psm.tile([128, BC], F32, tag="tp", name="tpv")
    nc.tensor.transpose(tpv[:, :], eout[:, :], identF[0:BC, 0:BC])
    vT = wtile([128, BC], BF16, "vT")
    nc.vector.tensor_copy(vT[:, :], tpv[:, :])
    cx01 = cxp.tile([128, BC], F32, tag="cx", name="cx01")
    nc.tensor.matmul(cx01[:, :], pack[:, 0:128], vT[:, :],
                     start=True, stop=True)
    cxs01 = wtile([128, BC], BF16, "cxs01")
    nc.scalar.activation(cxs01[:, :], cx01[:, :], AF.Relu, bias=mcb_col[:, :])
    cxhL = cxp.tile([64, BC], F32, tag="cx", name="cxhL")
    nc.tensor.matmul(cxhL[:, :], pack[:, 128:192], vT[:, :],
                     start=True, stop=True)
    nc.scalar.activation(cxshL[0:64, :], cxhL[:, :], AF.Relu,
                         bias=mcb_col[0:64, :])
    mp = psm.tile([BC, 256], F32, tag="mp", name="mp")
    nc.tensor.matmul(mp[:, :], cxs01[:, :], pack[:, 192:448],
                     start=True, stop=False)
    nc.tensor.matmul(mp[:, :], cxshL[:, :], pack[0:65, 448:704],
                     start=False, stop=True)

    # tokens = floor(|m|*100) mod 14.  floor(t) = round(t + (2^23-0.5)) - 2^23
    # (round-down magic; t<0.25 yields -0.5, fixed by the max clamp), then
    # two conditional subtracts for mod 14.
    tt = wtile([BC, 256], F32, "tt")
    nc.scalar.activation(tt[:, :], mp[:, :], AF.Abs, scale=100.0)
    fr = wtile([BC, 256], F32, "fr")
    nc.vector.tensor_scalar(fr[:, :], tt[:, :], 8388607.5, 8388608.0,
                            ALU.add, ALU.subtract)
    fc = wtile([BC, 256], F32, "fc")
    nc.vector.tensor_scalar(fc[:, :], fr[:, :], 0.0, None, ALU.max)
    ti = wtile([BC, 256], F32, "ti")
    nc.vector.tensor_scalar(ti[:, :], fc[:, :], float(V), None, ALU.is_ge)
    t1 = wtile([BC, 256], F32, "t1")
    nc.vector.scalar_tensor_tensor(t1[:, :], ti[:, :], -float(V), fc[:, :],
                                   ALU.mult, ALU.add)
    t2 = wtile([BC, 256], F32, "t2")
    nc.vector.tensor_scalar(t2[:, :], t1[:, :], float(V), None, ALU.is_ge)
    tok = wtile([BC, 256], F32, "tok")
    nc.vector.scalar_tensor_tensor(tok[:, :], t2[:, :], -float(V), t1[:, :],
                                   ALU.mult, ALU.add)
    _tap(nc, io, "tok", tok[:, :])

    # canonical sym pair index via ALU min/max
    e_, o_ = tok[:, 0:256:2], tok[:, 1:256:2]
    lo_ = wtile([BC, H], F32, "lo_")
    nc.vector.tensor_tensor(lo_[:, :], e_, o_, ALU.min)
    hi_ = wtile([BC, H], F32, "hi_")
    nc.vector.tensor_tensor(hi_[:, :], e_, o_, ALU.max)
    q_ = wtile([BC, H], F32, "q_")
    nc.vector.tensor_tensor(q_[:, :], lo_[:, :], lo_[:, :], ALU.mult)
    u1 = wtile([BC, H], F32, "u1")
    nc.vector.scalar_tensor_tensor(u1[:, :], lo_[:, :], 13.5, hi_[:, :],
                                   ALU.mult, ALU.add)
    idxF = wtile([BC, H], F32, "idxF")
    nc.vector.scalar_tensor_tensor(idxF[:, :], q_[:, :], -0.5, u1[:, :],
                                   ALU.mult, ALU.add)
    _tap(nc, io, "idxF", idxF[:, :])

    # transpose to h-major, flatten in 4 chunks (earlier pp0 start)
    tpF = psm.tile([128, BC], F32, tag="tp", name="tpF")
    nc.tensor.transpose(tpF[:, :], idxF[:, :], identF[0:BC, 0:BC])
    idxFT = wtile([128, BC], BF16, "idxFT")
    nc.vector.tensor_copy(idxFT[:, :], tpF[:, :])
    idxrowF = wtile([1, BC * H], BF16, "idxrowF")
    for c in range(4):
        nc.sync.dma_start(idxrowF[:, c * 1024:(c + 1) * 1024],
                          idxFT[c * 32:(c + 1) * 32, :])

    # ---------------- friend branch ----------------
    lpF = pacc.tile([128, 14], F32, tag="lp", name="lpF")
    branch(idxrowF, lambda k, hh: GF[:, (k * 16 + hh) * 14:(k * 16 + hh + 1) * 14],
           14, lpF, "F", 4, frontload=True)
    logitsF = combine4(lpF, 14, "F")
    ex = wtile([BC, 14], F32, "ex")
    sm = wtile([BC, 1], F32, "sm")
    nc.scalar.activation(ex[:, :], logitsF[:, :], AF.Exp, scale=float(inv_sf),
                         accum_out=sm[:, :])
    rs = wtile([BC, 1], F32, "rs")
    nc.vector.reciprocal(rs[:, :], sm[:, :])
    outt = wtile([BC, 14], F32, "outt")
    nc.vector.tensor_scalar(outt[:, :], ex[:, :], rs[:, :], None, ALU.mult)
    nc.sync.dma_start(io["out"], outt[:, :])


_CACHE = {}


def _get_nc(inv_se, inv_sf):
    key = ("nc", round(float(inv_se), 10), round(float(inv_sf), 10))
    if key not in _CACHE:
        nc = bacc.Bacc("TRN2", target_bir_lowering=False, debug=False,
                       num_devices=NCORES)
        with tile.TileContext(nc) as tc:
            with ExitStack() as ctx:
                build_kernel(nc, tc, ctx, inv_se, inv_sf)
        nc.compile()
        _CACHE[key] = nc
    return _CACHE[key]


def _build_G(emb, conv_w, lin_w, t0, t1, out_w=None):
    """G[p, h, j]: logits[b, j] = sum_h G[pid[b, h], h, j]."""
    f32 = np.float32
    emb = np.asarray(emb, f32)
    cw = np.ascontiguousarray(np.asarray(conv_w, f32)[:, :, :, 1])  # [O,I,3]
    table = np.maximum(emb[t0], emb[t1])                            # [P,512]
    CW = [table @ cw[:, :, kh].T for kh in range(3)]                # [P,256]
    W = np.asarray(lin_w, f32).reshape(256, H, -1)                  # [O,H,J]
    if out_w is not None:
        W = np.einsum("ohj,jn->ohn", W, np.asarray(out_w, f32))
    G = np.einsum("po,ohj->phj", CW[1], W)
    G[:, 0:H - 1] += np.einsum("po,ohj->phj", CW[0], W[:, 1:H])
    G[:, 1:H] += np.einsum("po,ohj->phj", CW[2], W[:, 0:H - 1])
    return G


def prep_inputs(inputs):
    """Host-side composition + shard. Returns (in_maps, inv_se, inv_sf)."""
    f32 = np.float32
    bf16 = ml_dtypes.bfloat16
    fp8 = ml_dtypes.float8_e4m3fn

    los, his = zip(*[(lo, hi) for lo in range(V) for hi in range(lo, V)])
    los, his = np.array(los), np.array(his)

    elw3 = np.asarray(inputs["enemy_lin_w"], f32).reshape(256, H, 128)
    elbe = (np.asarray(inputs["enemy_lin_b"], f32)
            + np.einsum("o,ohj->j", np.asarray(inputs["enemy_conv_b"], f32),
                        elw3, optimize=True))
    GE = _build_G(inputs["enemy_emb"], inputs["enemy_conv_w"],
                  inputs["enemy_lin_w"], los, his)
    GE += elbe[None, None, :] / H
    se = 240.0 / float(np.abs(GE).max())
    GEq = (GE * se).reshape(NSYM, 8, 2048).astype(fp8)
    GEck = np.empty((4 * NSYM, 4096), fp8)
    for c in range(4):
        GEck[c * NSYM:(c + 1) * NSYM, 0:2048] = GEq[:, 2 * c]
        GEck[c * NSYM:(c + 1) * NSYM, 2048:4096] = GEq[:, 2 * c + 1]

    flw3 = np.asarray(inputs["friend_lin1_w"], f32).reshape(256, H, 128)
    f2w = np.asarray(inputs["friend_lin2_w"], f32)
    flbe = (np.asarray(inputs["friend_lin1_b"], f32)
            + np.einsum("o,ohj->j", np.asarray(inputs["friend_conv_b"], f32),
                        flw3, optimize=True))
    f2be = flbe @ f2w + np.asarray(inputs["friend_lin2_b"], f32)
    GF = _build_G(inputs["friend_emb"], inputs["friend_conv_w"],
                  inputs["friend_lin1_w"], los, his, out_w=f2w)
    GF += f2be[None, None, :] / H
    sf = 240.0 / float(np.abs(GF).max())
    GFq = np.ascontiguousarray((GF * sf).reshape(NSYM, H * 14)).astype(fp8)

    mcw = np.asarray(inputs["manip_conv_w"], f32)[:, :, :, 1]  # [64,128,3]
    wsumT = np.concatenate([mcw.sum(2).T, (mcw[:, :, 1] + mcw[:, :, 2]).T],
                           axis=1)                              # [128, 128]
    ws_hL = (mcw[:, :, 0] + mcw[:, :, 1]).T                     # [128, 64]
    mlw3 = np.asarray(inputs["manip_lin_w"], f32).reshape(64, 128, 256)
    mlwA = np.concatenate([mlw3[:, 1:127].sum(1), mlw3[:, 0]], axis=0)
    mlwB = np.concatenate([mlw3[:, 127],
                           np.asarray(inputs["manip_lin_b"], f32)[None, :],
                           np.zeros((63, 256), f32)], axis=0)
    pack = np.concatenate(
        [wsumT, ws_hL, mlwA, mlwB], axis=1).astype(bf16)        # [128, 704]
    mcb = np.asarray(inputs["manip_conv_b"], f32)

    common = {
        "GE": np.ascontiguousarray(GEck), "GF": GFq,
        "pack": np.ascontiguousarray(pack),
        "mcb2": np.ascontiguousarray(np.concatenate([mcb, mcb])),
    }
    x = np.asarray(inputs["x"], np.int64)
    xlo = np.minimum(x[:, 0::2], x[:, 1::2])
    xhi = np.maximum(x[:, 0::2], x[:, 1::2])
    pid = (xhi + xlo * (27 - xlo) // 2).astype(bf16)   # [B, 128] ints < 105
    maps = []
    for c in range(NCORES):
        rowE = np.ascontiguousarray(
            pid[c * BC:(c + 1) * BC].T.reshape(1, BC * H))      # h-major
        maps.append(dict(common, idxrowE=rowE))
    return maps, 1.0 / se, 1.0 / sf


def kernel(**inputs):
    in_maps, inv_se, inv_sf = prep_inputs(inputs)
    nc = _get_nc(inv_se, inv_sf)
    res = run_bass_kernel_spmd(nc, in_maps, core_ids=list(range(NCORES)))
    return np.concatenate([r["out"] for r in res.results], axis=0)


# revision 27
# speedup vs baseline: 1.0093x; 1.0093x over previous
"""Trainium2 Bass kernel for nn_Network_67388036874689.

Data-parallel over batch: B=256 as 32 samples on each of 8 cores; params
replicated (host-precomposed).

Structure exploited (validated against the reference on host):
  - fog_of_war's greedy scan returns arange(B) -> permutation is identity.
  - conv2d(3x3, pad=1) on [C, H, 1] only sees kernel column 1 -> 1D 3-tap
    conv over H.
  - THE BIG FUSION: embedding + pair-maxpool + conv + linear collapse into
    a single gathered table per branch:
        logits[b, j] = sum_h G[pid[b, h], h, j]
    with G[p, h, j] = sum_kh CW_kh[p, :] @ W[:, h+1-kh, j] (host-built,
    boundary-clipped), pid = canonical unordered pair index (105 rows,
    pair-max is symmetric).  Linear bias folds in as bias[j]/128.
  - On device each branch: one-hot(pid) built 512 cols at a time
    (ones-matmul broadcast + is_equal vs iota), then 128 accumulating
    matmuls lpsum[32, J] += OH[:, h]^T @ G[:, h, :], 4-way col-tiled
    (tile_position=(0,32*(h%4))) so 4 streams share the PE array.
  - Both tables fp8 e4m3 (scaled; descale folded into the Exp scales):
    host sim rel err ~1.2e-3 vs 2e-2 tolerance.  The per-core DMA path
    sustains only ~200 GB/s (8 cores share HBM), so bytes are the
    enemy-phase currency: GE is streamed as 4 chunks with 4KB rows
    (per-descriptor packet overhead ~100ns amortizes with row length),
    in consumption order on the sync ring.
  - Manipulator fully bf16 (host sim 1.2e-3): conv input constant over
    h -> collapses to [128,192] wsumT + stacked [128/64, 256] linears.
  - floor via the 2^23 round-down magic (+max clamp for t<0.25); mod 14
    via 2 conditional subtracts; friend pair index = canonical sym index
    via ALU min/max: idx = hi + 13.5*lo - 0.5*lo^2.
"""

import numpy as np
import ml_dtypes
from contextlib import ExitStack

import concourse.bass as bass
import concourse.bacc as bacc
import concourse.mybir as mybir
import concourse.tile as tile
from concourse.masks import make_identity
from concourse.bass_utils import run_bass_kernel_spmd

F32 = mybir.dt.float32
F32R = mybir.dt.float32r
BF16 = mybir.dt.bfloat16
FP8 = mybir.dt.float8e4
I32 = mybir.dt.int32
AF = mybir.ActivationFunctionType
ALU = mybir.AluOpType
AX = mybir.AxisListType

NCORES = 8
B = 256
BC = B // NCORES        # 32 samples per core
L = 256
V = 14
H = L // 2              # 128 pooled positions
NSYM = V * (V + 1) // 2  # 105 canonical pairs
DEBUG_TAPS = False


def _dram_inputs(nc):
    t = {}

    def inp(name, shape, dt):
        t[name] = nc.dram_tensor(name, list(shape), dt, kind="ExternalInput").ap()

    inp("idxrowE", (1, BC * H), BF16)    # h-major: col = h*32 + b
    inp("GE", (4 * NSYM, 4096), FP8)     # chunk-major: chunk c rows hold
                                         # GE blocks 2c | 2c+1 (4KB rows)
    inp("GF", (NSYM, H * 14), FP8)       # col = h*14 + n, bias folded, *sF
    inp("pack", (128, 704), BF16)        # wsumT[0:192] mlwA[192:448]
                                         # mlwB[448:704] (rows 64+ zero)
    inp("mcb2", (128,), F32)
    t["out"] = nc.dram_tensor("out", [BC, 14], F32, kind="ExternalOutput").ap()
    return t


def _tap(nc, io, name, ap):
    if not DEBUG_TAPS:
        return
    t = nc.dram_tensor("tap_" + name, list(ap.shape), ap.dtype,
                       kind="ExternalOutput").ap()
    io["tap_" + name] = t
    nc.gpsimd.dma_start(t, ap)


def build_kernel(nc, tc, ctx, inv_se, inv_sf):
    io = _dram_inputs(nc)
    consts = ctx.enter_context(tc.tile_pool(name="consts", bufs=1))
    work = ctx.enter_context(tc.tile_pool(name="work", bufs=1))
    gepool = ctx.enter_context(tc.tile_pool(name="gepool", bufs=4))
    ohpool = ctx.enter_context(tc.tile_pool(name="ohpool", bufs=8))
    ppp = ctx.enter_context(tc.tile_pool(name="ppp", bufs=3, space="PSUM"))
    pacc = ctx.enter_context(tc.tile_pool(name="pacc", bufs=1, space="PSUM"))
    psm = ctx.enter_context(tc.tile_pool(name="psm", bufs=1, space="PSUM"))
    cxp = ctx.enter_context(tc.tile_pool(name="cxp", bufs=2, space="PSUM"))

    def ctile(shape, dt, tag):
        return consts.tile(shape, dt, tag=tag, name=tag)

    def wtile(shape, dt, tag):
        return work.tile(shape, dt, tag=tag, name=tag)

    # ---------------- constants ----------------
    identF = ctile([128, 128], F32, "identF")
    make_identity(nc, identF)
    iota_i = ctile([128, 1], I32, "iota_i")
    nc.gpsimd.iota(iota_i[:, :], pattern=[[0, 1]], base=0, channel_multiplier=1)
    iota_col = ctile([128, 1], F32, "iota_col")
    nc.vector.tensor_copy(iota_col[:, :], iota_i[:, :])
    ones_row = ctile([1, NSYM], BF16, "ones_row")
    nc.vector.memset(ones_row[:, :], 1.0)
    cxshL = ctile([65, BC], BF16, "cxshL")
    nc.vector.memset(cxshL[64:65, :], 1.0)   # mlb row of the augmented GEMM

    # sync ring, consumption order: idx row, GE chunks, GF
    idxrowE = wtile([1, BC * H], BF16, "idxrowE")
    nc.sync.dma_start(idxrowE[:, :], io["idxrowE"])
    GEc = []
    for c in range(4):
        t = gepool.tile([NSYM, 4096], FP8, tag="gec", name=f"gec{c}")
        nc.sync.dma_start(t[:, :], io["GE"][c * NSYM:(c + 1) * NSYM, :])
        GEc.append(t)
    GF = ctile([NSYM, H * 14], FP8, "GF")
    nc.sync.dma_start(GF[:, :], io["GF"])

    # small weights on the gpsimd ring
    mcb_col = ctile([128, 1], F32, "mcb2")
    nc.gpsimd.dma_start(mcb_col[:, :], io["mcb2"])
    pack = ctile([128, 704], BF16, "pack")
    nc.gpsimd.dma_start(pack[:, :], io["pack"])

    # ---------------- shared branch machinery ----------------
    def branch(idxrow, Gslice, J, lptile, tag, ways, frontload=False):
        """For each 16-h block: broadcast idx (PE), one-hot (DVE is_eq,
        fp8), then 16 gather matmuls col-tiled `ways`-wide into lptile.
        frontload issues all 8 pp+is_eq pairs before any gather so the
        one-hot build overlaps the table DMA window instead of trailing
        the gathers in PE program order."""
        def onehot(k):
            pp = ppp.tile([NSYM, 512], F32, tag="pp", name="pp")
            nc.tensor.matmul(pp[:, :], ones_row[:, :],
                             idxrow[:, k * 512:(k + 1) * 512],
                             start=True, stop=True)
            oh = ohpool.tile([NSYM, 512], FP8, tag="oh", name=f"oh{tag}{k}")
            nc.vector.tensor_scalar(oh[:, :], pp[:, :], iota_col[0:NSYM, :],
                                    None, ALU.is_equal)
            return oh

        def gathers(k, oh):
            for hh in range(16):
                h = k * 16 + hh
                j = h % ways
                nc.tensor.matmul(
                    lptile[32 * j:32 * (j + 1), 0:J],
                    oh[:, hh * 32:(hh + 1) * 32],
                    Gslice(k, hh),
                    start=(h < ways), stop=(h >= 128 - ways),
                    tile_position=(0, 32 * j), skip_group_check=True)

        if frontload:
            ohs = [onehot(k) for k in range(8)]
            for k in range(8):
                gathers(k, ohs[k])
        else:
            for k in range(8):
                gathers(k, onehot(k))

    def combine4(lptile, J, tag):
        """Sum the 4 col-tile partials (one PSUM read per DVE op)."""
        c1 = wtile([BC, J], F32, f"c1{tag}")
        nc.vector.tensor_copy(c1[:, :], lptile[32:64, 0:J])
        c2 = wtile([BC, J], F32, f"c2{tag}")
        nc.vector.tensor_copy(c2[:, :], lptile[64:96, 0:J])
        c3 = wtile([BC, J], F32, f"c3{tag}")
        nc.vector.tensor_copy(c3[:, :], lptile[96:128, 0:J])
        a01 = wtile([BC, J], F32, f"a01{tag}")
        nc.vector.tensor_tensor(a01[:, :], lptile[0:32, 0:J], c1[:, :], ALU.add)
        a23 = wtile([BC, J], F32, f"a23{tag}")
        nc.vector.tensor_tensor(a23[:, :], c2[:, :], c3[:, :], ALU.add)
        lg = wtile([BC, J], F32, f"lg{tag}")
        nc.vector.tensor_tensor(lg[:, :], a01[:, :], a23[:, :], ALU.add)
        return lg

    # ---------------- enemy branch ----------------
    lpE = pacc.tile([128, 128], F32, tag="lp", name="lpE")
    branch(idxrowE,
           lambda k, hh: GEc[k // 2][:, (k % 2) * 2048 + hh * 128:
                                     (k % 2) * 2048 + (hh + 1) * 128],
           128, lpE, "E", 4, frontload=True)
    logitsE = combine4(lpE, 128, "E")
    _tap(nc, io, "logitsE", logitsE[:, :])
    ExE = wtile([BC, 128], F32, "ExE")
    smE = wtile([BC, 1], F32, "smE")
    nc.scalar.activation(ExE[:, :], logitsE[:, :], AF.Exp, scale=float(inv_se),
                         accum_out=smE[:, :])
    rsE = wtile([BC, 1], F32, "rsE")
    nc.vector.reciprocal(rsE[:, :], smE[:, :])
    eout = wtile([BC, 128], F32, "eout")
    nc.vector.tensor_scalar(eout[:, :], ExE[:, :], rsE[:, :], None, ALU.mult)
    _tap(nc, io, "eout", eout[:, :])

    # ---------------- manipulator (bf16) ----------------
    tpv = psm.tile([128, BC], F32, tag="tp", name="tpv")
    nc.tensor.transpose(tpv[:, :], eout[:, :], identF[0:BC, 0:BC])
    vT = wtile([128, BC], BF16, "vT")
    nc.vector.tensor_copy(vT[:, :], tpv[:, :])
    cx01 = cxp.tile([128, BC], F32, tag="cx", name="cx01")
    nc.tensor.matmul(cx01[:, :], pack[:, 0:128], vT[:, :],
                     start=True, stop=True)
    cxs01 = wtile([128, BC], BF16, "cxs01")
    nc.scalar.activation(cxs01[:, :], cx01[:, :], AF.Relu, bias=mcb_col[:, :])
    cxhL = cxp.tile([64, BC], F32, tag="cx", name="cxhL")
    nc.tensor.matmul(cxhL[:, :], pack[:, 128:192], vT[:, :],
                     start=True, stop=True)
    nc.scalar.activation(cxshL[0:64, :], cxhL[:, :], AF.Relu,
                         bias=mcb_col[0:64, :])
    mp = psm.tile([BC, 256], F32, tag="mp", name="mp")
    nc.tensor.matmul(mp[:, :], cxs01[:, :], pack[:, 192:448],
                     start=True, stop=False)
    nc.tensor.matmul(mp[:, :], cxshL[:, :], pack[0:65, 448:704],
                     start=False, stop=True)

    # tokens = floor(|m|*100) mod 14.  floor(t) = round(t + (2^23-0.5)) - 2^23
    # (round-down magic; t<0.25 yields -0.5, fixed by the max clamp), then
    # two conditional subtracts for mod 14.
    tt = wtile([BC, 256], F32, "tt")
    nc.scalar.activation(tt[:, :], mp[:, :], AF.Abs, scale=100.0)
    fr = wtile([BC, 256], F32, "fr")
    nc.vector.tensor_scalar(fr[:, :], tt[:, :], 8388607.5, 8388608.0,
                            ALU.add, ALU.subtract)
    fc = wtile([BC, 256], F32, "fc")
    nc.vector.tensor_scalar(fc[:, :], fr[:, :], 0.0, None, ALU.max)
    ti = wtile([BC, 256], F32, "ti")
    nc.vector.tensor_scalar(ti[:, :], fc[:, :], float(V), None, ALU.is_ge)
    t1 = wtile([BC, 256], F32, "t1")
    nc.vector.scalar_tensor_tensor(t1[:, :], ti[:, :], -float(V), fc[:, :],
                                   ALU.mult, ALU.add)
    t2 = wtile([BC, 256], F32, "t2")
    nc.vector.tensor_scalar(t2[:, :], t1[:, :], float(V), None, ALU.is_ge)
    tok = wtile([BC, 256], F32, "tok")
    nc.vector.scalar_tensor_tensor(tok[:, :], t2[:, :], -float(V), t1[:, :],
                                   ALU.mult, ALU.add)
    _tap(nc, io, "tok", tok[:, :])

    # canonical sym pair index via ALU min/max
    e_, o_ = tok[:, 0:256:2], tok[:, 1:256:2]
    lo_ = wtile([BC, H], F32, "lo_")
    nc.vector.tensor_tensor(lo_[:, :], e_, o_, ALU.min)
    hi_ = wtile([BC, H], F32, "hi_")
    nc.vector.tensor_tensor(hi_[:, :], e_, o_, ALU.max)
    q_ = wtile([BC, H], F32, "q_")
    nc.vector.tensor_tensor(q_[:, :], lo_[:, :], lo_[:, :], ALU.mult)
    u1 = wtile([BC, H], F32, "u1")
    nc.vector.scalar_tensor_tensor(u1[:, :], lo_[:, :], 13.5, hi_[:, :],
                                   ALU.mult, ALU.add)
    idxF = wtile([BC, H], F32, "idxF")
    nc.vector.scalar_tensor_tensor(idxF[:, :], q_[:, :], -0.5, u1[:, :],
                                   ALU.mult, ALU.add)
    _tap(nc, io, "idxF", idxF[:, :])

    # transpose to h-major, flatten in 4 chunks (earlier pp0 start)
    tpF = psm.tile([128, BC], F32, tag="tp", name="tpF")
    nc.tensor.transpose(tpF[:, :], idxF[:, :], identF[0:BC, 0:BC])
    idxFT = wtile([128, BC], BF16, "idxFT")
    nc.vector.tensor_copy(idxFT[:, :], tpF[:, :])
    idxrowF = wtile([1, BC * H], BF16, "idxrowF")
    for c in range(4):
        nc.sync.dma_start(idxrowF[:, c * 1024:(c + 1) * 1024],
                          idxFT[c * 32:(c + 1) * 32, :])

    # ---------------- friend branch ----------------
    lpF = pacc.tile([128, 14], F32, tag="lp", name="lpF")
    branch(idxrowF, lambda k, hh: GF[:, (k * 16 + hh) * 14:(k * 16 + hh + 1) * 14],
           14, lpF, "F", 4, frontload=True)
    logitsF = combine4(lpF, 14, "F")
    ex = wtile([BC, 14], F32, "ex")
    sm = wtile([BC, 1], F32, "sm")
    nc.scalar.activation(ex[:, :], logitsF[:, :], AF.Exp, scale=float(inv_sf),
                         accum_out=sm[:, :])
    rs = wtile([BC, 1], F32, "rs")
    nc.vector.reciprocal(rs[:, :], sm[:, :])
    outt = wtile([BC, 14], F32, "outt")
    nc.vector.tensor_scalar(outt[:, :], ex[:, :], rs[:, :], None, ALU.mult)
    nc.sync.dma_start(io["out"], outt[:, :])


_CACHE = {}


def _get_nc(inv_se, inv_sf):
    key = ("nc", round(float(inv_se), 10), round(float(inv_sf), 10))
    if key not in _CACHE:
        nc = bacc.Bacc("TRN2", target_bir_lowering=False, debug=False,
                       num_devices=NCORES)
        with tile.TileContext(nc) as tc:
            with ExitStack() as ctx:
                build_kernel(nc, tc, ctx, inv_se, inv_sf)
        nc.compile()
        _CACHE[key] = nc
    return _CACHE[key]


def _build_G(emb, conv_w, lin_w, t0, t1, out_w=None):
    """G[p, h, j]: logits[b, j] = sum_h G[pid[b, h], h, j]."""
    f32 = np.float32
    emb = np.asarray(emb, f32)
    cw = np.ascontiguousarray(np.asarray(conv_w, f32)[:, :, :, 1])  # [O,I,3]
    table = np.maximum(emb[t0], emb[t1])                            # [P,512]
    CW = [table @ cw[:, :, kh].T for kh in range(3)]                # [P,256]
    W = np.asarray(lin_w, f32).reshape(256, H, -1)                  # [O,H,J]
    if out_w is not None:
        W = np.einsum("ohj,jn->ohn", W, np.asarray(out_w, f32))
    G = np.einsum("po,ohj->phj", CW[1], W)
    G[:, 0:H - 1] += np.einsum("po,ohj->phj", CW[0], W[:, 1:H])
    G[:, 1:H] += np.einsum("po,ohj->phj", CW[2], W[:, 0:H - 1])
    return G


def prep_inputs(inputs):
    """Host-side composition + shard. Returns (in_maps, inv_se, inv_sf)."""
    f32 = np.float32
    bf16 = ml_dtypes.bfloat16
    fp8 = ml_dtypes.float8_e4m3fn

    los, his = zip(*[(lo, hi) for lo in range(V) for hi in range(lo, V)])
    los, his = np.array(los), np.array(his)

    elw3 = np.asarray(inputs["enemy_lin_w"], f32).reshape(256, H, 128)
    elbe = (np.asarray(inputs["enemy_lin_b"], f32)
            + np.einsum("o,ohj->j", np.asarray(inputs["enemy_conv_b"], f32),
                        elw3, optimize=True))
    GE = _build_G(inputs["enemy_emb"], inputs["enemy_conv_w"],
                  inputs["enemy_lin_w"], los, his)
    GE += elbe[None, None, :] / H
    se = 240.0 / float(np.abs(GE).max())
    GEq = (GE * se).reshape(NSYM, 8, 2048).astype(fp8)
    GEck = np.empty((4 * NSYM, 4096), fp8)
    for c in range(4):
        GEck[c * NSYM:(c + 1) * NSYM, 0:2048] = GEq[:, 2 * c]
        GEck[c * NSYM:(c + 1) * NSYM, 2048:4096] = GEq[:, 2 * c + 1]

    flw3 = np.asarray(inputs["friend_lin1_w"], f32).reshape(256, H, 128)
    f2w = np.asarray(inputs["friend_lin2_w"], f32)
    flbe = (np.asarray(inputs["friend_lin1_b"], f32)
            + np.einsum("o,ohj->j", np.asarray(inputs["friend_conv_b"], f32),
                        flw3, optimize=True))
    f2be = flbe @ f2w + np.asarray(inputs["friend_lin2_b"], f32)
    GF = _build_G(inputs["friend_emb"], inputs["friend_conv_w"],
                  inputs["friend_lin1_w"], los, his, out_w=f2w)
    GF += f2be[None, None, :] / H
    sf = 240.0 / float(np.abs(GF).max())
    GFq = np.ascontiguousarray((GF * sf).reshape(NSYM, H * 14)).astype(fp8)

    mcw = np.asarray(inputs["manip_conv_w"], f32)[:, :, :, 1]  # [64,128,3]
    wsumT = np.concatenate([mcw.sum(2).T, (mcw[:, :, 1] + mcw[:, :, 2]).T],
                           axis=1)                              # [128, 128]
    ws_hL = (mcw[:, :, 0] + mcw[:, :, 1]).T                     # [128, 64]
    mlw3 = np.asarray(inputs["manip_lin_w"], f32).reshape(64, 128, 256)
    mlwA = np.concatenate([mlw3[:, 1:127].sum(1), mlw3[:, 0]], axis=0)
    mlwB = np.concatenate([mlw3[:, 127],
                           np.asarray(inputs["manip_lin_b"], f32)[None, :],
                           np.zeros((63, 256), f32)], axis=0)
    pack = np.concatenate(
        [wsumT, ws_hL, mlwA, mlwB], axis=1).astype(bf16)        # [128, 704]
    mcb = np.asarray(inputs["manip_conv_b"], f32)

    common = {
        "GE": np.ascontiguousarray(GEck), "GF": GFq,
        "pack": np.ascontiguousarray(pack),
        "mcb2": np.ascontiguousarray(np.concatenate([mcb, mcb])),
    }
    x = np.asarray(inputs["x"], np.int64)
    xlo = np.minimum(x[:, 0::2], x[:, 1::2])
    xhi = np.maximum(x[:, 0::2], x[:, 1::2])
    pid = (xhi + xlo * (27 - xlo) // 2).astype(bf16)   # [B, 128] ints < 105
    maps = []
    for c in range(NCORES):
        rowE = np.ascontiguousarray(
            pid[c * BC:(c + 1) * BC].T.reshape(1, BC * H))      # h-major
        maps.append(dict(common, idxrowE=rowE))
    return maps, 1.0 / se, 1.0 / sf


def kernel(**inputs):
    in_maps, inv_se, inv_sf = prep_inputs(inputs)
    nc = _get_nc(inv_se, inv_sf)
    res = run_bass_kernel_spmd(nc, in_maps, core_ids=list(range(NCORES)))
    return np.concatenate([r["out"] for r in res.results], axis=0)
